# revision 29
# baseline (speedup 1.0000x reference)
"""Trainium2 Bass kernel v3 for nn_CategoricalDecoder (topk_masking).

Bin-sharded single-pass design: each core computes full logits for its
1024 bins (f32r 1-term), derives num = full-feature logp sum and
score = tail-feature logp sum for all 256 batch rows via one-hot
matmuls, packs (20-bit fixed-point score key | 12-bit quantized num)
into positive fp32 bit patterns, and max8 extracts the per-row local
top-8 candidates WITH their num payloads in one instruction. An 8KB
AllToAll flips to batch sharding; the receiving core thresholds at the
16th-largest key and computes both logsumexps from the decoded
payloads. No z gather, no second net pass.
"""

import numpy as np
from contextlib import ExitStack

import bass_rust as _br
import concourse.bass as bass
import concourse.bacc as bacc
import concourse.tile as tile
from concourse import mybir
from concourse.bass_utils import run_bass_kernel_spmd
from concourse.hw_specs import get_activation_tables

F32 = mybir.dt.float32
F32R = mybir.dt.float32r
BF16 = mybir.dt.bfloat16
I32 = mybir.dt.int32
U8 = mybir.dt.uint8
AF = mybir.ActivationFunctionType
ALU = mybir.AluOpType
AX = mybir.AxisListType

B, N, Lz, H, D, C = 256, 8192, 64, 256, 32, 16
DC = D * C
P = 8
NL = N // P
BL = B // P
K = 16
NEG = -1.0

# packing constants
KEY_OFF, KEY_SCALE = 24.0, 16384.0
NUM_LO, NUM_W = -140.0, 80.0
QS = 4095.0 / NUM_W
M1, M2 = -86.0, -72.0  # fixed logsumexp shifts (num / den)

# kz column offsets (64-partition tile): w1 first so the first DMA chunk
# covers the first h matmul.
O_W1, O_ZT = 0, H
KZ_COLS = NL + H

# k128 column offsets (128-partition tile); urgent columns first so the
# first DMA chunk covers everything the logits/lse path needs.
O_B1 = 0                     # [128,2] b1 per m (tiny chunk, DMA'd first)
O_B2 = O_B1 + 2              # [128,4] b2 per t
O_QOFF = O_B2 + 4            # [128,2] q-affine bias per bt
O_KOFF = O_QOFF + 2          # [128,2] key-affine bias per bt
O_ENB = O_KOFF + 2           # [128,1] f32: NUM_LO - M1 (e_n exp bias)
O_URG = O_ENB + 1            # end of tiny bias chunk
O_W2S = O_URG                # 8 x [128,128] f32r: (t,kk) -> (t*2+kk)*128
O_GSEL = O_W2S + 1024        # 4 x [128,32] f32r
O_SPLIT = O_GSEL + 128       # end of urgent chunk
O_OHS = O_SPLIT              # 8 x [128,128] f32r: (t,bt) -> (t*2+bt)*128
O_OHT = O_OHS + 1024         # 2 x [128,128] f32r: tail one-hot (rows 64:128)
O_COEF = O_OHT + 256         # [32,128] of -1
O_COEFT = O_COEF + 128       # [32,128], -1 on rows 28:32 only
K128_COLS = O_COEFT + 128


class _Bacc(bacc.Bacc):
    """Bacc pinning activations to the one table holding
    {Relu, Exp, Ln, Copy}, avoiding per-switch ACT_TABLE_LOADs."""

    def insert_act_table_loads(self):
        has_act = any(isinstance(i, mybir.InstActivation)
                      for b in self.main_func.blocks for i in b.instructions)
        if not has_act:
            return
        tables = []
        for name, funcs in get_activation_tables(self.m.arch).items():
            keep = funcs if name == "natural_log_exp_and_others" else set()
            tables.append((name, keep))
        _br.insert_act_table_loads(self, tables)


def _build_nc():
    nc = _Bacc("TRN2", target_bir_lowering=False, num_devices=P)

    dp = nc.declare_dram_parameter
    kz = dp("kz", [Lz, KZ_COLS], F32R, isOutput=False)
    k128 = dp("k128", [128, K128_COLS], F32R, isOutput=False)
    outp = dp("out", [BL], F32, isOutput=True)

    with tile.TileContext(nc) as tc, ExitStack() as ctx:
        const = ctx.enter_context(tc.tile_pool(name="const", bufs=1))
        dram = ctx.enter_context(tc.tile_pool(name="dram", bufs=1, space="DRAM"))

        kz_sb = const.tile([Lz, KZ_COLS], F32R, name="kz_sb")
        nc.sync.dma_start(kz_sb[:, 0:H + 512], kz[:, 0:H + 512])
        nc.sync.dma_start(kz_sb[:, H + 512:], kz[:, H + 512:])
        k128_sb = const.tile([128, K128_COLS], F32R, name="k128_sb")
        nc.sync.dma_start(k128_sb[:, 0:O_URG], k128[:, 0:O_URG])
        nc.sync.dma_start(k128_sb[:, O_URG:O_SPLIT], k128[:, O_URG:O_SPLIT])
        nc.sync.dma_start(k128_sb[:, O_SPLIT:], k128[:, O_SPLIT:])

        def c128(off, w, p=128, dt=None):
            ap = k128_sb[0:p, off:off + w]
            return ap.bitcast(dt) if dt else ap

        xin = dram.tile([B, 8], F32)
        xout = dram.tile([B, 8], F32)
        act = ctx.enter_context(tc.tile_pool(name="act", bufs=1))
        scr = ctx.enter_context(tc.tile_pool(name="scr", bufs=6))

        # dummy activation: forces the ACT_TABLE_LOAD to run during the
        # parameter-DMA window instead of serializing with the first relu.
        warm = act.tile([1, 2], F32, name="warm")
        nc.vector.memset(warm[:], 0.0)
        nc.scalar.activation(warm[:], warm[:], AF.Relu)

        # ---- h = relu(W1.T @ zT + b1): separate [128,512] tiles per
        # (m, f) so the f=0 logits matmuls depend only on the f=0 relus.
        hrt = {}
        for m in range(2):
            for f in range(2):
                hrt[(m, f)] = act.tile([128, 512], F32R, name=f"hr{m}{f}")
        with ExitStack() as ctxh:
            hp = ctxh.enter_context(tc.tile_pool(name="hp", bufs=4, space="PSUM"))
            for f in range(2):
                for m in range(2):
                    ph = hp.tile([128, 512], F32, tag="h", name=f"ph{f}{m}")
                    nc.tensor.matmul(ph[:],
                                     kz_sb[:, O_W1 + m * 128:O_W1 + (m + 1) * 128],
                                     kz_sb[:, O_ZT + f * 512:O_ZT + (f + 1) * 512],
                                     start=True, stop=True)
                    nc.scalar.activation(hrt[(m, f)][:], ph[:], AF.Relu,
                                         bias=c128(O_B1 + m, 1, dt=F32))

        # ---- per f-chunk: logits, lse, num, score, pack ----
        packed = [act.tile([128, NL], I32, name=f"pk{bt}") for bt in range(2)]
        c12_t = act.tile([128, 512], I32, name="c12_t")
        nc.vector.memset(c12_t[:], 12)
        c4096_t = act.tile([128, 512], I32, name="c4096_t")
        nc.gpsimd.memset(c4096_t[:], 4096)
        xvh = [act.tile([128, 16], F32, name=f"xvh{bt}") for bt in range(2)]
        with ExitStack() as ctxA:
            lgp = ctxA.enter_context(tc.tile_pool(name="lgp", bufs=2, space="PSUM"))
            nump = ctxA.enter_context(tc.tile_pool(name="nump", bufs=4, space="PSUM"))
            psep = ctxA.enter_context(tc.tile_pool(name="psep", bufs=2, space="PSUM"))

            for f in range(2):
                sl = slice(f * 512, (f + 1) * 512)
                pnum = [nump.tile([128, 512], F32, tag="nm", name=f"pn{f}{i}")
                        for i in range(2)]
                pscore = [nump.tile([128, 512], F32, tag="nm", name=f"ps{f}{i}")
                          for i in range(2)]
                pse = psep.tile([32, 512], F32, tag="se")
                for ti, t in enumerate([3, 0, 1, 2]):
                    lg = lgp.tile([128, 512], F32, tag="lg")
                    for kk in range(2):
                        nc.tensor.matmul(
                            lg[:], c128(O_W2S + (t * 2 + kk) * 128, 128),
                            hrt[(kk, f)][:], start=(kk == 0), stop=(kk == 1))
                    e_t = scr.tile([128, 512], F32R, tag="e")
                    nc.scalar.activation(e_t[:], lg[:], AF.Exp,
                                         bias=c128(O_B2 + t, 1, dt=F32))
                    l_t = scr.tile([128, 512], F32R, tag="l")
                    nc.scalar.copy(l_t[:], lg[:])
                    nc.tensor.matmul(pse[:], c128(O_GSEL + t * 32, 32), e_t[:],
                                     start=(ti == 0), stop=(ti == 3))
                    for bt in range(2):
                        nc.tensor.matmul(pnum[bt][:],
                                         c128(O_OHS + (t * 2 + bt) * 128, 128),
                                         l_t[:], start=(ti == 0), stop=False)
                    if t == 3:
                        for bt in range(2):
                            nc.tensor.matmul(pscore[bt][:],
                                             c128(O_OHT + bt * 128, 128),
                                             l_t[:], start=True, stop=False)
                lnp = scr.tile([32, 512], F32R, tag="ln")
                nc.scalar.activation(lnp[:], pse[:], AF.Ln)
                for bt in range(2):
                    nc.tensor.matmul(pscore[bt][:], c128(O_COEFT, 128, p=32),
                                     lnp[:], start=False, stop=True)
                for bt in range(2):
                    nc.tensor.matmul(pnum[bt][:], c128(O_COEF, 128, p=32),
                                     lnp[:], start=False, stop=True)
                # pack: q = int(pnum*QS + qoff), key = int(pscore*KS + koff),
                # packed = key*4096 + q  (int32 domain)
                for bt in range(2):
                    q_i = scr.tile([128, 512], I32, tag="qi")
                    nc.vector.tensor_scalar(q_i[:], pnum[bt][:], QS,
                                            c128(O_QOFF + bt, 1, dt=F32),
                                            op0=ALU.mult, op1=ALU.add)
                    k_i = scr.tile([128, 512], I32, tag="ki")
                    nc.vector.tensor_scalar(k_i[:], pscore[bt][:], KEY_SCALE,
                                            c128(O_KOFF + bt, 1, dt=F32),
                                            op0=ALU.mult, op1=ALU.add)
                    k4 = scr.tile([128, 512], I32, tag="k4")
                    if bt == 0:
                        nc.gpsimd.tensor_tensor(k4[:], k_i[:], c4096_t[:],
                                                op=ALU.mult)
                        nc.gpsimd.tensor_tensor(packed[bt][:, sl], k4[:],
                                                q_i[:], op=ALU.add)
                    else:
                        nc.vector.tensor_tensor(k4[:], k_i[:], c12_t[:],
                                                op=ALU.logical_shift_left)
                        nc.vector.tensor_tensor(packed[bt][:, sl], k4[:],
                                                q_i[:], op=ALU.bitwise_or)
                    nc.vector.max(xvh[bt][:, f * 8:(f + 1) * 8],
                                  packed[bt][:, sl].bitcast(F32))

        # ---- local top-8 by packed value; ship via AllToAll ----
        xv = act.tile([128, 16], F32, name="xv")
        for bt in range(2):
            nc.vector.max(xv[:, bt * 8:(bt + 1) * 8], xvh[bt][:])
            nc.sync.dma_start(xin[bt * 128:(bt + 1) * 128, :],
                              xv[:, bt * 8:(bt + 1) * 8])

        nc.gpsimd.collective_compute(
            "AllToAll", ALU.bypass, replica_groups=[list(range(P))],
            ins=[xin[:].opt()], outs=[xout[:].opt()],
        )

        # ---- merge: threshold at 16th largest, masked logsumexps ----
        y3 = act.tile([BL, P, 8], F32, name="y3")
        nc.sync.dma_start(y3[:], xout[:].rearrange("(s r) c -> r s c", s=P))
        y = y3[:].rearrange("r s c -> r (s c)")
        w1 = act.tile([BL, 8], F32, name="w1")
        nc.vector.max(w1[:], y)
        y2 = act.tile([BL, P * 8], F32, name="y2")
        nc.vector.match_replace(y2[:], w1[:], y, NEG)
        w2 = act.tile([BL, 8], F32, name="w2")
        nc.vector.max(w2[:], y2[:])

        u_i = y.bitcast(I32)
        cfff_t = act.tile([BL, P * 8], I32, name="cfff_t")
        nc.vector.memset(cfff_t[:], 0xFFF)
        q_d = act.tile([BL, P * 8], I32, name="qd")
        nc.vector.tensor_tensor(q_d[:], u_i, cfff_t[:], op=ALU.bitwise_and)
        km_i = act.tile([BL, P * 8], I32, name="km")
        nc.vector.tensor_tensor(km_i[:], u_i, q_d[:], op=ALU.subtract)
        q_f = act.tile([BL, P * 8], F32, name="qf")
        nc.vector.tensor_copy(q_f[:], q_d[:])
        km_f = act.tile([BL, P * 8], F32, name="kmf")
        nc.vector.tensor_copy(km_f[:], km_i[:])

        # e_n = exp(num - M1), num = q/QS + NUM_LO
        e_n = act.tile([BL, P * 8], F32, name="en")
        nc.scalar.activation(e_n[:], q_f[:], AF.Exp,
                             scale=1.0 / QS, bias=c128(O_ENB, 1, p=BL, dt=F32))
        # den - M2 = q/QS - km/(4096*KS) + (NUM_LO + KEY_OFF - M2)
        nd = act.tile([BL, P * 8], F32, name="nd")
        nc.vector.tensor_scalar(nd[:], q_f[:], 1.0 / QS,
                                float(NUM_LO + KEY_OFF - M2),
                                op0=ALU.mult, op1=ALU.add)
        dd = act.tile([BL, P * 8], F32, name="dd")
        nc.vector.scalar_tensor_tensor(
            dd[:], km_f[:], -1.0 / (4096.0 * KEY_SCALE), nd[:],
            op0=ALU.mult, op1=ALU.add)
        e_d = act.tile([BL, P * 8], F32, name="ed")
        nc.scalar.activation(e_d[:], dd[:], AF.Exp)

        s2 = act.tile([BL, 2], F32, name="s2")
        junk = act.tile([BL, P * 8], F32, name="junk")
        nc.vector.scalar_tensor_tensor(
            junk[:], y, w2[:, 7:8], e_n[:],
            op0=ALU.is_ge, op1=ALU.mult, accum_out=s2[:, 0:1])
        junk2 = act.tile([BL, P * 8], F32, name="junk2")
        nc.vector.scalar_tensor_tensor(
            junk2[:], y, w2[:, 7:8], e_d[:],
            op0=ALU.is_ge, op1=ALU.mult, accum_out=s2[:, 1:2])
        lgt = act.tile([BL, 2], F32, name="lgt")
        nc.scalar.activation(lgt[:], s2[:], AF.Ln)
        res = act.tile([BL, 1], F32, name="res")
        nc.vector.scalar_tensor_tensor(
            res[:], lgt[:, 0:1], float(M1 - M2), lgt[:, 1:2],
            op0=ALU.add, op1=ALU.subtract)
        nc.sync.dma_start(outp[:], res[:, 0])

    nc.compile()
    return nc


def _tr12(a):
    a = np.ascontiguousarray(a, np.float32)
    return (a.view(np.uint32) & np.uint32(0xFFFFF000)).view(np.float32)


def _host_prep(x, z, W1, b1, W2, b2):
    # one-hot of x over the DC=512 logit rows
    oh = np.zeros((B, DC), np.float32)
    oh[np.arange(B)[:, None], np.arange(D)[None, :] * C + x] = 1.0
    ohT = np.ascontiguousarray(oh.T)                    # (512, 256)
    cbt = oh @ b2                                       # (256,)
    cbt_tail = oh[:, DC - 4 * C:] @ b2[DC - 4 * C:]     # (256,)

    k128c = np.zeros((128, K128_COLS), np.float32)
    w2t = _tr12(W2)
    for t in range(4):
        for kk in range(2):
            k128c[:, O_W2S + (t * 2 + kk) * 128:O_W2S + (t * 2 + kk + 1) * 128] = \
                w2t[kk * 128:(kk + 1) * 128, t * 128:(t + 1) * 128]
        for bt in range(2):
            k128c[:, O_OHS + (t * 2 + bt) * 128:O_OHS + (t * 2 + bt + 1) * 128] = \
                ohT[t * 128:(t + 1) * 128, bt * 128:(bt + 1) * 128]
    # tail one-hot: logits tile t=3, rows 64:128 are dc 448..511
    for bt in range(2):
        blk = np.zeros((128, 128), np.float32)
        blk[64:128, :] = ohT[448:512, bt * 128:(bt + 1) * 128]
        k128c[:, O_OHT + bt * 128:O_OHT + (bt + 1) * 128] = blk
    for t in range(4):
        g = np.zeros((128, 32), np.float32)
        g[np.arange(128), (t * 128 + np.arange(128)) // C] = 1.0
        k128c[:, O_GSEL + t * 32:O_GSEL + (t + 1) * 32] = g
    k128c[0:32, O_COEF:O_COEF + 128] = -1.0
    k128c[28:32, O_COEFT:O_COEFT + 128] = -1.0
    k128c[:, O_B1:O_B1 + 2] = b1.reshape(2, 128).T
    k128c[:, O_B2:O_B2 + 4] = b2.reshape(4, 128).T
    for bt in range(2):
        cb = cbt[bt * 128:(bt + 1) * 128]
        cbt4 = cbt_tail[bt * 128:(bt + 1) * 128]
        k128c[:, O_QOFF + bt] = (cb - NUM_LO) * QS + 0.5
        k128c[:, O_KOFF + bt] = (cbt4 + KEY_OFF) * KEY_SCALE + 0.5
    k128c[:, O_ENB] = NUM_LO - M1

    w1t = _tr12(W1)
    in_maps = []
    for c in range(P):
        kzc = np.zeros((Lz, KZ_COLS), np.float32)
        kzc[:, O_ZT:O_ZT + NL] = _tr12(z[c * NL:(c + 1) * NL, :].T)
        kzc[:, O_W1:O_W1 + H] = w1t
        in_maps.append(dict(kz=kzc, k128=k128c))
    return in_maps


_NC_CACHE = {}


def kernel(x, log_w, z, k, W1, b1, W2, b2, _trace=False, _trace_kwargs=None):
    assert int(k) == K
    in_maps = _host_prep(np.asarray(x, np.int32), np.asarray(z, np.float32),
                         np.asarray(W1, np.float32), np.asarray(b1, np.float32),
                         np.asarray(W2, np.float32), np.asarray(b2, np.float32))
    if "nc" not in _NC_CACHE:
        _NC_CACHE["nc"] = _build_nc()
    nc = _NC_CACHE["nc"]
    res = run_bass_kernel_spmd(
        nc, in_maps, list(range(P)), trace=_trace, **(_trace_kwargs or {}))
    if _trace:
        _NC_CACHE["last_result"] = res
    return np.concatenate([np.asarray(res.results[c]["out"], np.float32)
                           for c in range(P)])


# revision 30
# speedup vs baseline: 2.2230x; 2.2230x over previous
"""Trainium2 Bass kernel v3 for nn_CategoricalDecoder (topk_masking).

Bin-sharded single-pass design: each core computes full logits for its
1024 bins (f32r 1-term), derives num = full-feature logp sum and
score = tail-feature logp sum for all 256 batch rows via one-hot
matmuls, packs (20-bit fixed-point score key | 12-bit quantized num)
into positive fp32 bit patterns, and max8 extracts the per-row local
top-8 candidates WITH their num payloads in one instruction. An 8KB
AllToAll flips to batch sharding; the receiving core thresholds at the
16th-largest key and computes both logsumexps from the decoded
payloads. No z gather, no second net pass.
"""

import numpy as np
from contextlib import ExitStack

import bass_rust as _br
import concourse.bass as bass
import concourse.bacc as bacc
import concourse.tile as tile
from concourse import mybir
from concourse.bass_utils import run_bass_kernel_spmd
from concourse.hw_specs import get_activation_tables

F32 = mybir.dt.float32
F32R = mybir.dt.float32r
BF16 = mybir.dt.bfloat16
I32 = mybir.dt.int32
U8 = mybir.dt.uint8
AF = mybir.ActivationFunctionType
ALU = mybir.AluOpType
AX = mybir.AxisListType

B, N, Lz, H, D, C = 256, 8192, 64, 256, 32, 16
DC = D * C
P = 8
NL = N // P
BL = B // P
K = 16
NEG = -1.0

# packing constants
KEY_OFF, KEY_SCALE = 24.0, 16384.0
NUM_LO, NUM_W = -140.0, 80.0
QS = 4095.0 / NUM_W
M1, M2 = -86.0, -72.0  # fixed logsumexp shifts (num / den)

# kz column offsets (64-partition tile): w1 first so the first DMA chunk
# covers the first h matmul.
O_W1, O_ZT = 0, H
KZ_COLS = NL + H

# k128 column offsets (128-partition tile); urgent columns first so the
# first DMA chunk covers everything the logits/lse path needs.
O_B1 = 0                     # [128,2] b1 per m (tiny chunk, DMA'd first)
O_B2 = O_B1 + 2              # [128,4] b2 per t
O_QOFF = O_B2 + 4            # [128,2] q-affine bias per bt
O_KOFF = O_QOFF + 2          # [128,2] key-affine bias per bt
O_ENB = O_KOFF + 2           # [128,1] f32: NUM_LO - M1 (e_n exp bias)
O_URG = O_ENB + 1            # end of tiny bias chunk
O_W2S = O_URG                # 8 x [128,128] f32r: (t,kk) -> (t*2+kk)*128
O_GSEL = O_W2S + 1024        # 4 x [128,32] f32r
O_SPLIT = O_GSEL + 128       # end of urgent chunk
O_OHS = O_SPLIT              # 8 x [128,128] f32r: (t,bt) -> (t*2+bt)*128
O_OHT = O_OHS + 1024         # 2 x [128,128] f32r: tail one-hot (rows 64:128)
O_COEF = O_OHT + 256         # [32,128] of -1
O_COEFT = O_COEF + 128       # [32,128], -1 on rows 28:32 only
K128_COLS = O_COEFT + 128


class _Bacc(bacc.Bacc):
    """Bacc pinning activations to the one table holding
    {Relu, Exp, Ln, Copy}, avoiding per-switch ACT_TABLE_LOADs."""

    def insert_act_table_loads(self):
        has_act = any(isinstance(i, mybir.InstActivation)
                      for b in self.main_func.blocks for i in b.instructions)
        if not has_act:
            return
        tables = []
        for name, funcs in get_activation_tables(self.m.arch).items():
            keep = funcs if name == "natural_log_exp_and_others" else set()
            tables.append((name, keep))
        _br.insert_act_table_loads(self, tables)


def _build_nc():
    nc = _Bacc("TRN2", target_bir_lowering=False, num_devices=P)

    dp = nc.declare_dram_parameter
    kz = dp("kz", [Lz, KZ_COLS], F32R, isOutput=False)
    k128 = dp("k128", [128, K128_COLS], F32R, isOutput=False)
    outp = dp("out", [BL], F32, isOutput=True)

    with tile.TileContext(nc) as tc, ExitStack() as ctx:
        const = ctx.enter_context(tc.tile_pool(name="const", bufs=1))
        dram = ctx.enter_context(tc.tile_pool(name="dram", bufs=1, space="DRAM"))

        kz_sb = const.tile([Lz, KZ_COLS], F32R, name="kz_sb")
        nc.sync.dma_start(kz_sb[:, 0:H + 512], kz[:, 0:H + 512])
        nc.sync.dma_start(kz_sb[:, H + 512:], kz[:, H + 512:])
        k128_sb = const.tile([128, K128_COLS], F32R, name="k128_sb")
        nc.sync.dma_start(k128_sb[:, 0:O_URG], k128[:, 0:O_URG])
        nc.sync.dma_start(k128_sb[:, O_URG:O_SPLIT], k128[:, O_URG:O_SPLIT])
        nc.sync.dma_start(k128_sb[:, O_SPLIT:], k128[:, O_SPLIT:])

        def c128(off, w, p=128, dt=None):
            ap = k128_sb[0:p, off:off + w]
            return ap.bitcast(dt) if dt else ap

        xin = dram.tile([B, 8], F32)
        xout = dram.tile([B, 8], F32)
        act = ctx.enter_context(tc.tile_pool(name="act", bufs=1))
        scr = ctx.enter_context(tc.tile_pool(name="scr", bufs=6))

        # dummy activation: forces the ACT_TABLE_LOAD to run during the
        # parameter-DMA window instead of serializing with the first relu.
        warm = act.tile([1, 2], F32, name="warm")
        nc.vector.memset(warm[:], 0.0)
        nc.scalar.activation(warm[:], warm[:], AF.Relu)

        # ---- h = relu(W1.T @ zT + b1): separate [128,512] tiles per
        # (m, f) so the f=0 logits matmuls depend only on the f=0 relus.
        hrt = {}
        for m in range(2):
            for f in range(2):
                hrt[(m, f)] = act.tile([128, 512], F32R, name=f"hr{m}{f}")
        with ExitStack() as ctxh:
            hp = ctxh.enter_context(tc.tile_pool(name="hp", bufs=4, space="PSUM"))
            for f in range(2):
                for m in range(2):
                    ph = hp.tile([128, 512], F32, tag="h", name=f"ph{f}{m}")
                    nc.tensor.matmul(ph[:],
                                     kz_sb[:, O_W1 + m * 128:O_W1 + (m + 1) * 128],
                                     kz_sb[:, O_ZT + f * 512:O_ZT + (f + 1) * 512],
                                     start=True, stop=True)
                    nc.scalar.activation(hrt[(m, f)][:], ph[:], AF.Relu,
                                         bias=c128(O_B1 + m, 1, dt=F32))

        # ---- per f-chunk: logits, lse, num, score, pack ----
        packed = [act.tile([128, NL], I32, name=f"pk{bt}") for bt in range(2)]
        c12_t = act.tile([128, 512], I32, name="c12_t")
        nc.vector.memset(c12_t[:], 12)
        c4096_t = act.tile([128, 512], I32, name="c4096_t")
        nc.gpsimd.memset(c4096_t[:], 4096)
        xvh = [act.tile([128, 16], F32, name=f"xvh{bt}") for bt in range(2)]
        with ExitStack() as ctxA:
            lgp = ctxA.enter_context(tc.tile_pool(name="lgp", bufs=2, space="PSUM"))
            nump = ctxA.enter_context(tc.tile_pool(name="nump", bufs=4, space="PSUM"))
            psep = ctxA.enter_context(tc.tile_pool(name="psep", bufs=2, space="PSUM"))

            for f in range(2):
                sl = slice(f * 512, (f + 1) * 512)
                pnum = [nump.tile([128, 512], F32, tag="nm", name=f"pn{f}{i}")
                        for i in range(2)]
                pscore = [nump.tile([128, 512], F32, tag="nm", name=f"ps{f}{i}")
                          for i in range(2)]
                pse = psep.tile([32, 512], F32, tag="se")
                for ti, t in enumerate([3, 0, 1, 2]):
                    lg = lgp.tile([128, 512], F32, tag="lg")
                    for kk in range(2):
                        nc.tensor.matmul(
                            lg[:], c128(O_W2S + (t * 2 + kk) * 128, 128),
                            hrt[(kk, f)][:], start=(kk == 0), stop=(kk == 1))
                    e_t = scr.tile([128, 512], F32R, tag="e")
                    nc.scalar.activation(e_t[:], lg[:], AF.Exp,
                                         bias=c128(O_B2 + t, 1, dt=F32))
                    l_t = scr.tile([128, 512], F32R, tag="l")
                    nc.scalar.copy(l_t[:], lg[:])
                    nc.tensor.matmul(pse[:], c128(O_GSEL + t * 32, 32), e_t[:],
                                     start=(ti == 0), stop=(ti == 3))
                    for bt in range(2):
                        nc.tensor.matmul(pnum[bt][:],
                                         c128(O_OHS + (t * 2 + bt) * 128, 128),
                                         l_t[:], start=(ti == 0), stop=False)
                    if t == 3:
                        for bt in range(2):
                            nc.tensor.matmul(pscore[bt][:],
                                             c128(O_OHT + bt * 128, 128),
                                             l_t[:], start=True, stop=False)
                lnp = scr.tile([32, 512], F32R, tag="ln")
                nc.scalar.activation(lnp[:], pse[:], AF.Ln)
                for bt in range(2):
                    nc.tensor.matmul(pnum[bt][:], c128(O_COEF, 128, p=32),
                                     lnp[:], start=False, stop=True)
                    nc.tensor.matmul(pscore[bt][:], c128(O_COEFT, 128, p=32),
                                     lnp[:], start=False, stop=True)
                # pack: q = int(pnum*QS + qoff), key = int(pscore*KS + koff),
                # packed = key*4096 + q  (int32 domain)
                for bt in range(2):
                    q_i = scr.tile([128, 512], I32, tag="qi")
                    k_i = scr.tile([128, 512], I32, tag="ki")
                    if bt == 0:
                        nc.scalar.activation(q_i[:], pnum[bt][:], AF.Relu,
                                             bias=c128(O_QOFF + bt, 1, dt=F32),
                                             scale=QS)
                        nc.scalar.activation(k_i[:], pscore[bt][:], AF.Relu,
                                             bias=c128(O_KOFF + bt, 1, dt=F32),
                                             scale=KEY_SCALE)
                    else:
                        nc.vector.tensor_scalar(q_i[:], pnum[bt][:], QS,
                                                c128(O_QOFF + bt, 1, dt=F32),
                                                op0=ALU.mult, op1=ALU.add)
                        nc.vector.tensor_scalar(k_i[:], pscore[bt][:],
                                                KEY_SCALE,
                                                c128(O_KOFF + bt, 1, dt=F32),
                                                op0=ALU.mult, op1=ALU.add)
                    k4 = scr.tile([128, 512], I32, tag="k4")
                    if bt == 0:
                        nc.gpsimd.tensor_tensor(k4[:], k_i[:], c4096_t[:],
                                                op=ALU.mult)
                        nc.gpsimd.tensor_tensor(packed[bt][:, sl], k4[:],
                                                q_i[:], op=ALU.add)
                    else:
                        nc.vector.tensor_tensor(k4[:], k_i[:], c12_t[:],
                                                op=ALU.logical_shift_left)
                        nc.vector.tensor_tensor(packed[bt][:, sl], k4[:],
                                                q_i[:], op=ALU.bitwise_or)
                    nc.vector.max(xvh[bt][:, f * 8:(f + 1) * 8],
                                  packed[bt][:, sl].bitcast(F32))

        # ---- local top-8 by packed value; ship via AllToAll ----
        xv = act.tile([128, 16], F32, name="xv")
        for bt in range(2):
            nc.vector.max(xv[:, bt * 8:(bt + 1) * 8], xvh[bt][:])
            nc.sync.dma_start(xin[bt * 128:(bt + 1) * 128, :],
                              xv[:, bt * 8:(bt + 1) * 8])

        nc.gpsimd.collective_compute(
            "AllToAll", ALU.bypass, replica_groups=[list(range(P))],
            ins=[xin[:].opt()], outs=[xout[:].opt()],
        )

        # ---- merge: threshold at 16th largest, masked logsumexps ----
        y3 = act.tile([BL, P, 8], F32, name="y3")
        nc.sync.dma_start(y3[:], xout[:].rearrange("(s r) c -> r s c", s=P))
        y = y3[:].rearrange("r s c -> r (s c)")
        w1 = act.tile([BL, 8], F32, name="w1")
        nc.vector.max(w1[:], y)
        y2 = act.tile([BL, P * 8], F32, name="y2")
        nc.vector.match_replace(y2[:], w1[:], y, NEG)
        w2 = act.tile([BL, 8], F32, name="w2")
        nc.vector.max(w2[:], y2[:])

        u_i = y.bitcast(I32)
        cfff_t = act.tile([BL, P * 8], I32, name="cfff_t")
        nc.vector.memset(cfff_t[:], 0xFFF)
        q_d = act.tile([BL, P * 8], I32, name="qd")
        nc.vector.tensor_tensor(q_d[:], u_i, cfff_t[:], op=ALU.bitwise_and)
        km_i = act.tile([BL, P * 8], I32, name="km")
        nc.vector.tensor_tensor(km_i[:], u_i, q_d[:], op=ALU.subtract)
        q_f = act.tile([BL, P * 8], F32, name="qf")
        nc.vector.tensor_copy(q_f[:], q_d[:])
        km_f = act.tile([BL, P * 8], F32, name="kmf")
        nc.vector.tensor_copy(km_f[:], km_i[:])

        # e_n = exp(num - M1), num = q/QS + NUM_LO
        e_n = act.tile([BL, P * 8], F32, name="en")
        nc.scalar.activation(e_n[:], q_f[:], AF.Exp,
                             scale=1.0 / QS, bias=c128(O_ENB, 1, p=BL, dt=F32))
        # den - M2 = q/QS - km/(4096*KS) + (NUM_LO + KEY_OFF - M2)
        nd = act.tile([BL, P * 8], F32, name="nd")
        nc.vector.tensor_scalar(nd[:], q_f[:], 1.0 / QS,
                                float(NUM_LO + KEY_OFF - M2),
                                op0=ALU.mult, op1=ALU.add)
        dd = act.tile([BL, P * 8], F32, name="dd")
        nc.vector.scalar_tensor_tensor(
            dd[:], km_f[:], -1.0 / (4096.0 * KEY_SCALE), nd[:],
            op0=ALU.mult, op1=ALU.add)
        e_d = act.tile([BL, P * 8], F32, name="ed")
        nc.scalar.activation(e_d[:], dd[:], AF.Exp)

        s2 = act.tile([BL, 2], F32, name="s2")
        junk = act.tile([BL, P * 8], F32, name="junk")
        nc.vector.scalar_tensor_tensor(
            junk[:], y, w2[:, 7:8], e_n[:],
            op0=ALU.is_ge, op1=ALU.mult, accum_out=s2[:, 0:1])
        junk2 = act.tile([BL, P * 8], F32, name="junk2")
        nc.vector.scalar_tensor_tensor(
            junk2[:], y, w2[:, 7:8], e_d[:],
            op0=ALU.is_ge, op1=ALU.mult, accum_out=s2[:, 1:2])
        lgt = act.tile([BL, 2], F32, name="lgt")
        nc.scalar.activation(lgt[:], s2[:], AF.Ln)
        res = act.tile([BL, 1], F32, name="res")
        nc.vector.scalar_tensor_tensor(
            res[:], lgt[:, 0:1], float(M1 - M2), lgt[:, 1:2],
            op0=ALU.add, op1=ALU.subtract)
        nc.sync.dma_start(outp[:], res[:, 0])

    nc.compile()
    return nc


def _tr12(a):
    a = np.ascontiguousarray(a, np.float32)
    return (a.view(np.uint32) & np.uint32(0xFFFFF000)).view(np.float32)


def _host_prep(x, z, W1, b1, W2, b2):
    # one-hot of x over the DC=512 logit rows
    oh = np.zeros((B, DC), np.float32)
    oh[np.arange(B)[:, None], np.arange(D)[None, :] * C + x] = 1.0
    ohT = np.ascontiguousarray(oh.T)                    # (512, 256)
    cbt = oh @ b2                                       # (256,)
    cbt_tail = oh[:, DC - 4 * C:] @ b2[DC - 4 * C:]     # (256,)

    k128c = np.zeros((128, K128_COLS), np.float32)
    w2t = _tr12(W2)
    for t in range(4):
        for kk in range(2):
            k128c[:, O_W2S + (t * 2 + kk) * 128:O_W2S + (t * 2 + kk + 1) * 128] = \
                w2t[kk * 128:(kk + 1) * 128, t * 128:(t + 1) * 128]
        for bt in range(2):
            k128c[:, O_OHS + (t * 2 + bt) * 128:O_OHS + (t * 2 + bt + 1) * 128] = \
                ohT[t * 128:(t + 1) * 128, bt * 128:(bt + 1) * 128]
    # tail one-hot: logits tile t=3, rows 64:128 are dc 448..511
    for bt in range(2):
        blk = np.zeros((128, 128), np.float32)
        blk[64:128, :] = ohT[448:512, bt * 128:(bt + 1) * 128]
        k128c[:, O_OHT + bt * 128:O_OHT + (bt + 1) * 128] = blk
    for t in range(4):
        g = np.zeros((128, 32), np.float32)
        g[np.arange(128), (t * 128 + np.arange(128)) // C] = 1.0
        k128c[:, O_GSEL + t * 32:O_GSEL + (t + 1) * 32] = g
    k128c[0:32, O_COEF:O_COEF + 128] = -1.0
    k128c[28:32, O_COEFT:O_COEFT + 128] = -1.0
    k128c[:, O_B1:O_B1 + 2] = b1.reshape(2, 128).T
    k128c[:, O_B2:O_B2 + 4] = b2.reshape(4, 128).T
    for bt in range(2):
        cb = cbt[bt * 128:(bt + 1) * 128]
        cbt4 = cbt_tail[bt * 128:(bt + 1) * 128]
        k128c[:, O_QOFF + bt] = (cb - NUM_LO) * QS + 0.5
        k128c[:, O_KOFF + bt] = (cbt4 + KEY_OFF) * KEY_SCALE + 0.5
    k128c[:, O_ENB] = NUM_LO - M1

    w1t = _tr12(W1)
    in_maps = []
    for c in range(P):
        kzc = np.zeros((Lz, KZ_COLS), np.float32)
        kzc[:, O_ZT:O_ZT + NL] = _tr12(z[c * NL:(c + 1) * NL, :].T)
        kzc[:, O_W1:O_W1 + H] = w1t
        in_maps.append(dict(kz=kzc, k128=k128c))
    return in_maps


_NC_CACHE = {}


def kernel(x, log_w, z, k, W1, b1, W2, b2, _trace=False, _trace_kwargs=None):
    assert int(k) == K
    in_maps = _host_prep(np.asarray(x, np.int32), np.asarray(z, np.float32),
                         np.asarray(W1, np.float32), np.asarray(b1, np.float32),
                         np.asarray(W2, np.float32), np.asarray(b2, np.float32))
    if "nc" not in _NC_CACHE:
        _NC_CACHE["nc"] = _build_nc()
    nc = _NC_CACHE["nc"]
    res = run_bass_kernel_spmd(
        nc, in_maps, list(range(P)), trace=_trace, **(_trace_kwargs or {}))
    if _trace:
        _NC_CACHE["last_result"] = res
    return np.concatenate([np.asarray(res.results[c]["out"], np.float32)
                           for c in range(P)])
